# revision 1
# baseline (speedup 1.0000x reference)
"""Trainium2 Bass kernel for nn_Attention_decoder (conv + linear-attention + convT block).

Math refactoring (validated vs reference to ~3e-6 rel err):
  - All BatchNorms folded into weights/biases (eval mode, affine).
  - No softmax => the two N^2 einsums collapse by associativity:
      m1 = vf^T @ (qf @ k) = (vf^T qf) @ k = A @ k,  A is 128x128.
  - out-proj and out1 folded:  xo = (Wo1a@out_w/16 @ A) @ k + Wo1b @ x_conv + bias.
  - ConvTranspose2d(k=3,s=2,p=1,op=1) decomposed into 4 parity sub-convs with
    (da,db) in {0,1}^2 offsets over the (bottom/right zero-padded) input.

Sharding: 8 cores = 4 batches x 2 row-halves. Per (batch, half) core:
  - conv1 on a 35-row window (33 out rows: the half + 1 halo row) -> xb_win
  - conv1 on the OTHER half's 34-row window (32 out rows) -> xb_rest (only to
    complete the spatially-global attention Gram matrix A = vf^T qf)
  - A accumulated over all 64 rows; QT = A^T-ish @ RT; k, xo on the 33-row
    window; convT parity grids on the 32 local a-rows -> 64 output rows.
All matmuls run as float32r (fp32 data, FP22 multiply, fp32 accumulate) at
full 1 cycle/row PE throughput for moving dims >= 256.
"""

import os
import sys

for _p in ("/opt/trn_rl_repo", "/root/.axon_site/_ro/trn_rl_repo"):
    if os.path.isdir(_p) and _p not in sys.path:
        sys.path.insert(0, _p)

import numpy as np

import concourse.bass as bass
import concourse.mybir as mybir
import concourse.tile as tile
from concourse import bacc
from concourse.bass_utils import run_bass_kernel_spmd

EPS = 1e-5
B, C, H, W = 4, 256, 64, 64
CH = 128  # attention head dim
P = 128   # partitions
F32 = mybir.dt.float32
F32R = mybir.dt.float32r

WIN_IN = 35    # input rows for the window conv (33 out rows)
WIN_OUT = 33   # window out rows: 32 half rows + 1 halo
REST_IN = 34   # input rows for the rest conv (32 out rows)
REST_OUT = 32
WPAD = 66      # horizontally padded width

# convT parity grid taps: (r, s) -> [(ky, kx, da, db), ...]
#   out[2a+r, 2b+s] += sum_taps trw[:, :, ky, kx]^T @ xo[a+da, b+db]
CT_TAPS = {
    (0, 0): [(1, 1, 0, 0)],
    (0, 1): [(1, 0, 0, 1), (1, 2, 0, 0)],
    (1, 0): [(0, 1, 1, 0), (2, 1, 0, 0)],
    (1, 1): [(0, 0, 1, 1), (0, 2, 1, 0), (2, 0, 0, 1), (2, 2, 0, 0)],
}

LAST_EXEC_TIME_NS = None
LAST_PROFILE = None

_CACHE = {}


def _fold_bn(g, b, m, v):
    s = g / np.sqrt(v + EPS)
    return s.astype(np.float64), (b - m * s).astype(np.float64)


def _prep_weights(inp):
    """Host-side BN folding + layout prep. Returns dict of shared per-core arrays."""
    f8 = lambda a: np.asarray(a, np.float64)
    s1, t1 = _fold_bn(f8(inp["bn1_g"]), f8(inp["bn1_b"]), f8(inp["bn1_m"]), f8(inp["bn1_v"]))
    W1 = f8(inp["conv_w"]) * s1[:, None, None, None]          # (co, ci, ky, kx)
    B1 = s1 * f8(inp["conv_b"]) + t1                          # (256,)
    sq, tq = _fold_bn(f8(inp["qbn_g"]), f8(inp["qbn_b"]), f8(inp["qbn_m"]), f8(inp["qbn_v"]))
    Wq = f8(inp["q_w"]) * sq[:, None]
    Bq = sq * f8(inp["q_b"]) + tq
    sk, tk = _fold_bn(f8(inp["kbn_g"]), f8(inp["kbn_b"]), f8(inp["kbn_m"]), f8(inp["kbn_v"]))
    Wk = f8(inp["k_w"]) * sk[:, None]
    Bk = sk * f8(inp["k_b"]) + tk
    Wv = f8(inp["v_w"])
    Bv = f8(inp["v_b"])
    so, to = _fold_bn(f8(inp["obn_g"]), f8(inp["obn_b"]), f8(inp["obn_m"]), f8(inp["obn_v"]))
    Wo1 = f8(inp["out1_w"]) * so[:, None]                     # (256, 512)
    Bo1 = so * f8(inp["out1_b"]) + to
    Wo1a, Wo1b = Wo1[:, :C], Wo1[:, C:]
    Rm = Wo1a @ f8(inp["out_w"]) / 16.0                       # (256, 128)
    bias_xo = Bo1 + Wo1a @ f8(inp["out_b"])                   # (256,)
    trw = f8(inp["tr_w"])                                     # (ci, co, ky, kx)
    trb = f8(inp["tr_b"])

    d = {}
    # conv1 lhsT: w1t[a, o, t*2 + i, b] = W1[o*128+b, i*128+a, ky, kx]
    w1t = np.zeros((P, 2, 18, P), np.float32)
    trt = np.zeros((P, 2, 18, P), np.float32)
    for t in range(9):
        ky, kx = divmod(t, 3)
        for i in range(2):
            for o in range(2):
                idx = t * 2 + i
                w1t[:, o, idx, :] = W1[o*P:(o+1)*P, i*P:(i+1)*P, ky, kx].T
                trt[:, o, idx, :] = trw[i*P:(i+1)*P, o*P:(o+1)*P, ky, kx]
    d["w1t"] = w1t
    d["trt"] = trt
    d["b1"] = np.ascontiguousarray(B1.reshape(2, P).T.astype(np.float32))      # (128, 2)
    d["trb"] = np.ascontiguousarray(trb.reshape(2, P).T.astype(np.float32))    # (128, 2)
    # qv moving weights: qvt[a, i, 0:128]=Wq[c, i*128+a]; [...,128:256]=Wv
    qvt = np.zeros((P, 2, 2 * CH), np.float32)
    for i in range(2):
        qvt[:, i, :CH] = Wq[:, i*P:(i+1)*P].T
        qvt[:, i, CH:] = Wv[:, i*P:(i+1)*P].T
    d["qvt"] = qvt
    d["qvbias"] = np.broadcast_to(
        np.concatenate([Bq, Bv]).astype(np.float32)[None, :], (P, 2 * CH)
    ).copy()
    wkt = np.zeros((P, 2, CH), np.float32)
    for i in range(2):
        wkt[:, i, :] = Wk[:, i*P:(i+1)*P].T
    d["wkt"] = wkt
    d["bk"] = Bk.astype(np.float32).reshape(P, 1)
    d["rt"] = np.ascontiguousarray(Rm.T.astype(np.float32))                    # (128, 256)
    wo1bt = np.zeros((P, 2, C), np.float32)
    for i in range(2):
        wo1bt[:, i, :] = Wo1b[:, i*P:(i+1)*P].T
    d["wo1bt"] = wo1bt
    d["bxo"] = np.ascontiguousarray(bias_xo.reshape(2, P).T.astype(np.float32))  # (128, 2)
    return d


def _prep_core_inputs(inp, shared):
    """Per-core (batch b, half h) sliced + padded activations."""
    x = np.asarray(inp["x"], np.float32).reshape(B, 2, P, H, W)
    xc = np.asarray(inp["x_conv"], np.float32).reshape(B, 2, P, H, W)
    in_maps = []
    for core in range(8):
        b, h = divmod(core, 2)
        r0 = 32 * h
        # window: conv out rows [r0, r0+33) -> input rows [r0-1, r0+34), cols [-1, 65)
        xwin = np.zeros((2, P, WIN_IN, WPAD), np.float32)
        lo, hi = r0 - 1, r0 + WIN_IN - 1
        slo, shi = max(lo, 0), min(hi, H)
        xwin[:, :, slo - lo:slo - lo + (shi - slo), 1:W + 1] = x[b, :, :, slo:shi, :]
        # rest: other half, conv out rows [q0, q0+32) -> input rows [q0-1, q0+33)
        q0 = 32 * (1 - h)
        xrest = np.zeros((2, P, REST_IN, WPAD), np.float32)
        lo, hi = q0 - 1, q0 + REST_IN - 1
        slo, shi = max(lo, 0), min(hi, H)
        xrest[:, :, slo - lo:slo - lo + (shi - slo), 1:W + 1] = x[b, :, :, slo:shi, :]
        # x_conv rows [r0, r0+33), zero-padded past the image
        xconv = np.zeros((2, P, WIN_OUT, W), np.float32)
        shi = min(r0 + WIN_OUT, H)
        xconv[:, :, :shi - r0, :] = xc[b, :, :, r0:shi, :]
        m = dict(shared)
        m["xwin"] = xwin
        m["xrest"] = xrest
        m["xconv"] = xconv.reshape(2, P, WIN_OUT * W)
        m["lastmask"] = np.full((P, 1), 1.0 if h == 0 else 0.0, np.float32)
        m["zcol"] = np.zeros((P, WIN_OUT), np.float32)
        in_maps.append(m)
    return in_maps


def _build_program(cc=False):
    nc = bacc.Bacc(trn_type="TRN2", num_devices=8)

    # ---- DRAM I/O ----
    t_xwin = nc.dram_tensor("xwin", [2, P, WIN_IN, WPAD], F32R, kind="ExternalInput")
    t_xrest = None
    if not cc:
        t_xrest = nc.dram_tensor("xrest", [2, P, REST_IN, WPAD], F32R, kind="ExternalInput")
    t_xconv = nc.dram_tensor("xconv", [2, P, WIN_OUT * W], F32R, kind="ExternalInput")
    t_w1t = nc.dram_tensor("w1t", [P, 2, 18, P], F32R, kind="ExternalInput")
    t_trt = nc.dram_tensor("trt", [P, 2, 18, P], F32R, kind="ExternalInput")
    t_b1 = nc.dram_tensor("b1", [P, 2], F32, kind="ExternalInput")
    t_trb = nc.dram_tensor("trb", [P, 2], F32, kind="ExternalInput")
    t_qvt = nc.dram_tensor("qvt", [P, 2, 2 * CH], F32R, kind="ExternalInput")
    t_qvbias = nc.dram_tensor("qvbias", [P, 2 * CH], F32, kind="ExternalInput")
    t_wkt = nc.dram_tensor("wkt", [P, 2, CH], F32R, kind="ExternalInput")
    t_bk = nc.dram_tensor("bk", [P, 1], F32, kind="ExternalInput")
    t_rt = nc.dram_tensor("rt", [P, 2 * CH], F32R, kind="ExternalInput")
    t_wo1bt = nc.dram_tensor("wo1bt", [P, 2, C], F32R, kind="ExternalInput")
    t_bxo = nc.dram_tensor("bxo", [P, 2], F32, kind="ExternalInput")
    t_lastmask = nc.dram_tensor("lastmask", [P, 1], F32, kind="ExternalInput")
    t_zcol = nc.dram_tensor("zcol", [P, WIN_OUT], F32R, kind="ExternalInput")
    # out[o, p, a, r, col] = output row 2a+r (within the core's 64-row half)
    t_out = nc.dram_tensor("out", [2, P, 32, 2, 2 * W], F32, kind="ExternalOutput")

    WIN_CHUNKS = [(0, 7), (7, 7), (14, 7), (21, 6), (27, 6)]
    REST_CHUNKS = [(0, 8), (8, 8), (16, 8), (24, 8)]
    K_CHUNKS = [(0, 7), (7, 7), (14, 7), (21, 6), (27, 6)]
    XO_CHUNKS = [(0, 8), (8, 8), (16, 8), (24, 8), (32, 1)]

    with tile.TileContext(nc) as tc:
        with (
            tc.tile_pool(name="persist", bufs=1) as pp,
            tc.tile_pool(name="qv", bufs=4) as qvp,
            tc.tile_pool(name="line", bufs=4) as linep,
        ):
            # ---- critical-path loads first: conv1 weights + window input ----
            s_w1t = pp.tile([P, 2, 18, P], F32R, tag="w1t", name="s_w1t")
            s_b1 = pp.tile([P, 2], F32, tag="b1", name="s_b1")
            s_xwin = [pp.tile([P, WIN_IN, WPAD], F32R, tag=f"xwin{i}", name=f"s_xwin{i}") for i in range(2)]
            # chunked rows; order so conv chunk 0 (o=0) inputs land first
            XWIN_ROWS = [(0, 9), (9, 7), (16, 7), (23, 6), (29, 6)]
            for g0 in (0, 6, 12):
                nc.sync.dma_start(s_w1t[:, 0, g0:g0 + 6], t_w1t[:, 0, g0:g0 + 6])
            for i in range(2):
                nc.sync.dma_start(s_xwin[i][:, 0:9], t_xwin[i, :, 0:9])
            nc.sync.dma_start(s_b1[:], t_b1[:])
            nc.sync.dma_start(s_w1t[:, 1], t_w1t[:, 1])
            for (r0, nr) in XWIN_ROWS[1:]:
                for i in range(2):
                    nc.sync.dma_start(s_xwin[i][:, r0:r0 + nr], t_xwin[i, :, r0:r0 + nr])

            s_xbwin = [pp.tile([P, WIN_OUT * W], F32R, tag=f"xbwin{o}", name=f"s_xbwin{o}") for o in range(2)]
            s_xbrest = None
            if not cc:
                s_xbrest = [pp.tile([P, REST_OUT * W], F32R, tag=f"xbrest{o}", name=f"s_xbrest{o}") for o in range(2)]
            s_k = pp.tile([P, WIN_OUT * W], F32R, tag="k", name="s_k")
            s_A = pp.tile([P, CH], F32R, tag="A", name="s_A")
            s_QT = pp.tile([P, 2 * CH], F32R, tag="QT", name="s_QT")
            s_xopad = [pp.tile([P, WIN_OUT, WPAD - 1], F32R, tag=f"xopad{o}", name=f"s_xopad{o}") for o in range(2)]

            def conv1(src, dst, chunks):
                """src: [2][P, rows, WPAD] padded input; dst: [2][P, out_rows*64]."""
                for (a0, nr) in chunks:
                    for o in range(2):
                        ncols = nr * W
                        ps = psMM.tile([P, 512], F32, tag="mm", name="ps_mm")
                        n_mm = 18
                        mi = 0
                        for t in range(9):
                            ky, kx = divmod(t, 3)
                            for i in range(2):
                                rhs = src[i][:, a0 + ky:a0 + ky + nr, kx:kx + W]
                                nc.tensor.matmul(
                                    ps[:, :ncols],
                                    s_w1t[:, o, t * 2 + i, :],
                                    rhs,
                                    start=(mi == 0),
                                    stop=(mi == n_mm - 1),
                                )
                                mi += 1
                        nc.any.tensor_scalar_add(
                            dst[o][:, a0 * W:a0 * W + ncols], ps[:, :ncols],
                            s_b1[:, o:o + 1],
                        )

            with (
                tc.tile_pool(name="psMM", bufs=4, space="PSUM") as psMM,
                tc.tile_pool(name="psA", bufs=1, space="PSUM") as psA,
                tc.tile_pool(name="psQV", bufs=3, space="PSUM") as psQV,
            ):
                conv1(s_xwin, s_xbwin, WIN_CHUNKS)

                # ---- k = Wk @ xb_win + Bk (only needs the window) ----
                s_wkt = pp.tile([P, 2, CH], F32R, tag="wkt", name="s_wkt")
                nc.sync.dma_start(s_wkt[:], t_wkt[:])
                s_bk = pp.tile([P, 1], F32, tag="bk", name="s_bk")
                nc.sync.dma_start(s_bk[:], t_bk[:])
                for (a0, nr) in K_CHUNKS:
                    c0, ncols = a0 * W, nr * W
                    ps = psMM.tile([P, 512], F32, tag="mm", name="ps_mm")
                    for i in range(2):
                        nc.tensor.matmul(
                            ps[:, :ncols],
                            s_wkt[:, i, :],
                            s_xbwin[i][:, c0:c0 + ncols],
                            start=(i == 0),
                            stop=(i == 1),
                        )
                    nc.any.tensor_scalar_add(s_k[:, c0:c0 + ncols], ps[:, :ncols], s_bk[:])

                # ---- rest-window conv (completes the global Gram reduction) ----
                if not cc:
                    s_xrest = [pp.tile([P, REST_IN, WPAD], F32R, tag=f"xrest{i}", name=f"s_xrest{i}") for i in range(2)]
                    XREST_ROWS = [(0, 10), (10, 8), (18, 8), (26, 8)]
                    for (r0, nr) in XREST_ROWS:
                        for i in range(2):
                            nc.sync.dma_start(s_xrest[i][:, r0:r0 + nr], t_xrest[i, :, r0:r0 + nr])
                    conv1(s_xrest, s_xbrest, REST_CHUNKS)

                # ---- qT/vT chunks + A accumulation (A = v @ q^T, 128x128) ----
                s_qvt = pp.tile([P, 2, 2 * CH], F32R, tag="qvt", name="s_qvt")
                nc.sync.dma_start(s_qvt[:], t_qvt[:])
                s_qvbias = pp.tile([P, 2 * CH], F32, tag="qvbias", name="s_qvbias")
                nc.sync.dma_start(s_qvbias[:], t_qvbias[:])
                s_rt = pp.tile([P, 2 * CH], F32R, tag="rt", name="s_rt")
                nc.sync.dma_start(s_rt[:], t_rt[:])

                ps_A = psA.tile([P, CH], F32, tag="A", name="ps_A")
                n_chunks = 16 if cc else 32
                for cidx in range(n_chunks):
                    if cidx < 16:
                        xb = s_xbwin
                        col = cidx * P
                    else:
                        xb = s_xbrest
                        col = (cidx - 16) * P
                    ps_qv = psQV.tile([P, 2 * CH], F32, tag="qv", name="ps_qv")
                    for i in range(2):
                        nc.tensor.matmul(
                            ps_qv[:],
                            xb[i][:, col:col + P],
                            s_qvt[:, i, :],
                            start=(i == 0),
                            stop=(i == 1),
                        )
                    qv = qvp.tile([P, 2 * CH], F32R, tag="qv", name="qv_sb")
                    nc.any.tensor_tensor(qv[:], ps_qv[:], s_qvbias[:], mybir.AluOpType.add)
                    nc.tensor.matmul(
                        ps_A[:],
                        qv[:, CH:],      # lhsT = vT chunk [n, cv]
                        qv[:, :CH],      # rhs  = qT chunk [n, cq]
                        start=(cidx == 0),
                        stop=(cidx == n_chunks - 1),
                        skip_group_check=True,
                    )
                if cc:
                    s_Araw = pp.tile([P, CH], F32, tag="Araw", name="s_Araw")
                    nc.any.tensor_copy(s_Araw[:], ps_A[:])
                    with tc.tile_pool(name="ccdram", bufs=1, space="DRAM") as dramp:
                        cc_in = dramp.tile([P, CH], F32, name="cc_in")
                        cc_out = dramp.tile([P, CH], F32, name="cc_out")
                        nc.gpsimd.dma_start(cc_in[:], s_Araw[:])
                        nc.gpsimd.collective_compute(
                            "AllReduce",
                            mybir.AluOpType.add,
                            replica_groups=[[0, 1], [2, 3], [4, 5], [6, 7]],
                            ins=[cc_in.opt()],
                            outs=[cc_out.opt()],
                        )
                        nc.gpsimd.dma_start(s_Araw[:], cc_out[:])
                    # DVE copy rounds f32 -> f32r for the matmul consumer
                    nc.any.tensor_copy(s_A[:], s_Araw[:])
                else:
                    nc.any.tensor_copy(s_A[:], ps_A[:])

                # ---- QT = A^T @ RT  ([ck, co] lhsT for the xo matmuls) ----
                ps_qt = psQV.tile([P, 2 * CH], F32, tag="qv", name="ps_qt")
                nc.tensor.matmul(ps_qt[:], s_A[:], s_rt[:], start=True, stop=True)
                nc.any.tensor_copy(s_QT[:], ps_qt[:])

            # ---- phase 2: xo + convT with a deeper PSUM pool ----
            s_xconv = [pp.tile([P, WIN_OUT * W], F32R, tag=f"xconv{i}", name=f"s_xconv{i}") for i in range(2)]
            for i in range(2):
                nc.sync.dma_start(s_xconv[i][:], t_xconv[i])
            s_wo1bt = pp.tile([P, 2, C], F32R, tag="wo1bt", name="s_wo1bt")
            nc.sync.dma_start(s_wo1bt[:], t_wo1bt[:])
            s_bxo = pp.tile([P, 2], F32, tag="bxo", name="s_bxo")
            nc.sync.dma_start(s_bxo[:], t_bxo[:])
            s_lastmask = pp.tile([P, 1], F32, tag="lastmask", name="s_lastmask")
            nc.sync.dma_start(s_lastmask[:], t_lastmask[:])
            s_trt = pp.tile([P, 2, 18, P], F32R, tag="trt", name="s_trt")
            for o in range(2):
                nc.sync.dma_start(s_trt[:, o], t_trt[:, o])
            s_trb = pp.tile([P, 2], F32, tag="trb", name="s_trb")
            nc.sync.dma_start(s_trb[:], t_trb[:])
            for o in range(2):
                # right pad column must be zero (memset can't encode f32r)
                nc.sync.dma_start(s_xopad[o][:, :, W:W + 1], t_zcol[:, :, None])

            with tc.tile_pool(name="psMM2", bufs=6, space="PSUM") as psMM2:
                # ---- xo = QT^T k + Wo1b x_conv + bias (masked last halo row) ----
                for o in range(2):
                    for (a0, nr) in XO_CHUNKS:
                        c0, ncols = a0 * W, nr * W
                        ps = psMM2.tile([P, 512], F32, tag="mm2", name="ps_mm2")
                        nc.tensor.matmul(
                            ps[:, :ncols],
                            s_QT[:, o * CH:(o + 1) * CH],
                            s_k[:, c0:c0 + ncols],
                            start=True, stop=False,
                        )
                        for i in range(2):
                            nc.tensor.matmul(
                                ps[:, :ncols],
                                s_wo1bt[:, i, o * CH:(o + 1) * CH],
                                s_xconv[i][:, c0:c0 + ncols],
                                start=False, stop=(i == 1),
                            )
                        dst = s_xopad[o][:, a0:a0 + nr, 0:W]
                        src2 = ps[:, :ncols].rearrange("p (a w) -> p a w", w=W)
                        if a0 == 32:
                            # halo row: (x + bias) * mask (mask=0 on the bottom-half core)
                            nc.any.tensor_scalar(
                                dst, src2, s_bxo[:, o:o + 1], s_lastmask[:],
                                op0=mybir.AluOpType.add, op1=mybir.AluOpType.mult,
                            )
                        else:
                            nc.any.tensor_scalar_add(dst, src2, s_bxo[:, o:o + 1])

                # ---- convT: 4 parity grids over local a in [0, 32) ----
                for r in range(2):
                    for a0 in (0, 8, 16, 24):
                        for o in range(2):
                            line = linep.tile([P, 8, 2 * W], F32, tag="line", name="line")
                            for s in range(2):
                                taps = CT_TAPS[(r, s)]
                                ps = psMM2.tile([P, 512], F32, tag="mm2", name="ps_mm2")
                                n_mm = len(taps) * 2
                                mi = 0
                                for (ky, kx, da, db) in taps:
                                    t = ky * 3 + kx
                                    for i in range(2):
                                        rhs = s_xopad[i][:, a0 + da:a0 + da + 8, db:db + W]
                                        nc.tensor.matmul(
                                            ps[:],
                                            s_trt[:, o, t * 2 + i, :],
                                            rhs,
                                            start=(mi == 0),
                                            stop=(mi == n_mm - 1),
                                        )
                                        mi += 1
                                nc.any.tensor_scalar_add(
                                    line[:, :, s::2],
                                    ps.rearrange("p (a w) -> p a w", w=W),
                                    s_trb[:, o:o + 1],
                                )
                            nc.sync.dma_start(t_out[o, :, a0:a0 + 4, r, :], line[:, 0:4])
                            nc.gpsimd.dma_start(t_out[o, :, a0 + 4:a0 + 8, r, :], line[:, 4:8])

    nc.compile()
    return nc


def _ensure_ntff_hook():
    """antenv.axon_hooks is absent in this image; recreate it + install the
    ctypes NTFF hook so run_bass_kernel_spmd(trace=True) can profile."""
    try:
        from antenv import axon_hooks  # noqa: F401
        return
    except ImportError:
        pass
    try:
        import types
        import antenv
        mod = types.ModuleType("antenv.axon_hooks")
        _hook = [None]
        mod.set_axon_ntff_profile_hook = lambda h: _hook.__setitem__(0, h)
        mod.get_axon_ntff_profile_hook = lambda: _hook[0]
        sys.modules["antenv.axon_hooks"] = mod
        antenv.axon_hooks = mod
        from trn_agent_boot.trn_boot import _ntff_profile_via_ctypes
        mod.set_axon_ntff_profile_hook(
            _ntff_profile_via_ctypes("/opt/axon/libaxon_pjrt.so")
        )
    except Exception:
        pass


def kernel(**inputs):
    global LAST_EXEC_TIME_NS, LAST_PROFILE
    cc = os.environ.get("KERNEL_CC", "0") == "1"
    key = f"nc_cc{int(cc)}"
    if key not in _CACHE:
        _CACHE[key] = _build_program(cc=cc)
    nc = _CACHE[key]
    shared = _prep_weights(inputs)
    in_maps = _prep_core_inputs(inputs, shared)
    trace = os.environ.get("KERNEL_PROFILE", "") in ("1", "true")
    if trace:
        _ensure_ntff_hook()
    res = run_bass_kernel_spmd(nc, in_maps, core_ids=list(range(8)), trace=trace)
    LAST_EXEC_TIME_NS = getattr(res, "exec_time_ns", None)
    LAST_PROFILE = getattr(res, "profile_json", None)
    out = np.zeros((B, C, 2 * H, 2 * W), np.float32)
    for core in range(8):
        b, h = divmod(core, 2)
        o = res.results[core]["out"]  # (2, 128, 32, 2, 128)
        out[b, :, 64 * h:64 * (h + 1), :] = o.reshape(C, 64, 2 * W)
    return out


if __name__ == "__main__":
    rng = np.random.default_rng(0)
    fake = {}
    fake["x"] = rng.standard_normal((B, C, H, W), np.float32)
    fake["x_conv"] = rng.standard_normal((B, C, H, W), np.float32)
    print("smoke build only")
    _build_program()
    print("build ok")

